# revision 4
# baseline (speedup 1.0000x reference)
"""AFT-Full (Attention-Free Transformer) distributed Bass kernel for 8 TRN2 NeuronCores.

Reference math (B=4, N=512, D=128):
    q = sigmoid(x @ Wq^T + bq); k = x @ Wk^T + bk; v = x @ Wv^T + bv
    s[b,t,j,d] = k[b,j,d] + pos_bias[t,j];  m = max_j s
    out = q * (sum_j exp(s-m) * v) / (sum_j exp(s-m))

The max-stabilizer m cancels between numerator and denominator, and
exp(k + pb) = exp(pb) * exp(k), so with P = exp(pos_bias), ek = exp(k):
    out = q * (P @ (ek * v)) / (P @ ek)        (matmuls contract over j)
Values are O(1) (x ~ N(0,1), W ~ N(0,1)/sqrt(D), pb ~ 0.1*N(0,1)) so fp32
exp without the stabilizer is numerically safe.

Sharding: 8 cores = 4 batches x 2 t-halves; no collectives. Each core gets
x[b]^T with its t-half's columns rotated to the front, and pos_bias rows
rotated identically, so the j-contraction order matches and one SPMD graph
serves all cores. Device computes out^T[d, t] for its (b, t-half).
"""

import sys

import numpy as np

try:
    import concourse.bass as bass
except ImportError:  # pragma: no cover
    sys.path.insert(0, "/opt/trn_rl_repo")
    import concourse.bass as bass

import concourse.mybir as mybir
import concourse.tile as tile
from concourse import bacc
from concourse.bass_utils import run_bass_kernel_spmd

F32 = mybir.dt.float32
B, N, D = 4, 512, 128
T = N // 2  # t-rows per core
JT = N // 128  # j tiles of 128
AF = mybir.ActivationFunctionType


def build_nc() -> bass.Bass:
    # Bacc (not plain Bass): its compile() pass legalizes multi-wait
    # instructions (move_matmul_waits_to_ldweights, event semaphores),
    # which this walrus build requires.
    nc = bacc.Bacc()
    xT = nc.dram_tensor("xT", [D, N], F32, kind="ExternalInput")
    wkv = nc.dram_tensor("wkv", [D, 2 * D], F32, kind="ExternalInput")
    wqT = nc.dram_tensor("wqT", [D, D], F32, kind="ExternalInput")
    bkv = nc.dram_tensor("bkv", [1, 2 * D], F32, kind="ExternalInput")
    bq = nc.dram_tensor("bq", [1, D], F32, kind="ExternalInput")
    pbT = nc.dram_tensor("pbT", [N, T], F32, kind="ExternalInput")
    out = nc.dram_tensor("out", [D, T], F32, kind="ExternalOutput")

    with tile.TileContext(nc) as tc:
        with (
            tc.tile_pool(name="sb", bufs=1) as sb,
            tc.tile_pool(name="ps", bufs=2, space="PSUM") as ps,
            tc.tile_pool(name="acc", bufs=1, space="PSUM") as acc,
        ):
            xT_sb = sb.tile([D, N], F32, name="xT_sb")
            nc.sync.dma_start(xT_sb[:], xT[:])
            wkv_sb = sb.tile([D, 2 * D], F32, name="wkv_sb")
            nc.sync.dma_start(wkv_sb[:], wkv[:])
            wq_sb = sb.tile([D, D], F32, name="wq_sb")
            nc.sync.dma_start(wq_sb[:], wqT[:])
            bkv_sb = sb.tile([1, 2 * D], F32, name="bkv_sb")
            nc.sync.dma_start(bkv_sb[:], bkv[:])
            bq_sb = sb.tile([1, D], F32, name="bq_sb")
            nc.sync.dma_start(bq_sb[:], bq[:])
            ones_sb = sb.tile([1, T], F32, name="ones_sb")
            nc.vector.memset(ones_sb[:], 1.0)

            pb_sb = [sb.tile([128, T], F32, name=f"pb{j}") for j in range(JT)]
            for j in range(JT):
                nc.sync.dma_start(pb_sb[j][:], pbT[j * 128 : (j + 1) * 128, :])

            pt_sb = [sb.tile([128, T], F32, name=f"pt{j}") for j in range(JT)]
            ek_sb = [sb.tile([128, D], F32, name=f"ek{j}") for j in range(JT)]
            w_sb = [sb.tile([128, D], F32, name=f"w{j}") for j in range(JT)]

            for j in range(JT):
                nc.scalar.activation(pt_sb[j][:], pb_sb[j][:], AF.Exp)

            # k/v projections, one j-tile at a time: PSUM accumulates
            # (ones^T @ [bk|bv]) + (x_jtile @ [Wk^T|Wv^T]) = [k+bk | v+bv]
            for j in range(JT):
                kv_ps = ps.tile([128, 2 * D], F32, tag="kv_ps")
                nc.tensor.matmul(
                    kv_ps[:], ones_sb[:, 0:128], bkv_sb[:], start=True, stop=False
                )
                nc.tensor.matmul(
                    kv_ps[:],
                    xT_sb[:, j * 128 : (j + 1) * 128],
                    wkv_sb[:],
                    start=False,
                    stop=True,
                )
                nc.scalar.activation(ek_sb[j][:], kv_ps[:, 0:D], AF.Exp)
                nc.vector.tensor_mul(w_sb[j][:], ek_sb[j][:], kv_ps[:, D : 2 * D])

            # q^T[d, t] = sigmoid(bq broadcast + Wq @ x[t-half]^T)
            q_ps = acc.tile([D, T], F32, tag="q_ps")
            nc.tensor.matmul(q_ps[:], bq_sb[:], ones_sb[:], start=True, stop=False)
            nc.tensor.matmul(q_ps[:], wq_sb[:], xT_sb[:, 0:T], start=False, stop=True)
            qs_sb = sb.tile([D, T], F32, name="qs_sb")
            nc.scalar.activation(qs_sb[:], q_ps[:], AF.Sigmoid)

            # den^T[d,t] = sum_j ek[j,d] * P^T[j,t]; num^T with w = ek*v
            den_ps = acc.tile([D, T], F32, tag="den_ps")
            num_ps = acc.tile([D, T], F32, tag="num_ps")
            for j in range(JT):
                nc.tensor.matmul(
                    den_ps[:], ek_sb[j][:], pt_sb[j][:],
                    start=(j == 0), stop=(j == JT - 1),
                )
                nc.tensor.matmul(
                    num_ps[:], w_sb[j][:], pt_sb[j][:],
                    start=(j == 0), stop=(j == JT - 1),
                )

            rec_sb = sb.tile([D, T], F32, name="rec_sb")
            nc.vector.reciprocal(rec_sb[:], den_ps[:])
            t1_sb = sb.tile([D, T], F32, name="t1_sb")
            nc.vector.tensor_mul(t1_sb[:], num_ps[:], rec_sb[:])
            out_sb = sb.tile([D, T], F32, name="out_sb")
            nc.vector.tensor_mul(out_sb[:], t1_sb[:], qs_sb[:])
            nc.sync.dma_start(out[:], out_sb[:])

    nc.finalize()
    return nc


def prepare_in_maps(x, Wq, bq, Wk, bk, Wv, bv, pos_bias):
    x = np.asarray(x, dtype=np.float32)
    pos_bias = np.asarray(pos_bias, dtype=np.float32)
    wkv = np.ascontiguousarray(
        np.concatenate(
            [np.asarray(Wk, np.float32).T, np.asarray(Wv, np.float32).T], axis=1
        )
    )
    wqT = np.ascontiguousarray(np.asarray(Wq, np.float32).T)
    bkv_row = np.ascontiguousarray(
        np.concatenate([np.asarray(bk, np.float32), np.asarray(bv, np.float32)])[None]
    )
    bq_row = np.ascontiguousarray(np.asarray(bq, np.float32)[None])

    in_maps = []
    for i in range(8):
        b, th = divmod(i, 2)
        t0 = th * T
        perm = np.concatenate([np.arange(t0, N), np.arange(0, t0)])
        in_maps.append(
            {
                "xT": np.ascontiguousarray(x[b][perm].T),
                "wkv": wkv,
                "wqT": wqT,
                "bkv": bkv_row,
                "bq": bq_row,
                "pbT": np.ascontiguousarray(pos_bias[t0 : t0 + T][:, perm].T),
            }
        )
    return in_maps


def assemble_output(results) -> np.ndarray:
    out = np.empty((B, N, D), np.float32)
    for i in range(8):
        b, th = divmod(i, 2)
        t0 = th * T
        out[b, t0 : t0 + T, :] = results[i]["out"].T
    return out


def kernel(x, Wq, bq, Wk, bk, Wv, bv, pos_bias) -> np.ndarray:
    in_maps = prepare_in_maps(x, Wq, bq, Wk, bk, Wv, bv, pos_bias)
    nc = build_nc()
    res = run_bass_kernel_spmd(nc, in_maps, core_ids=list(range(8))).results
    return assemble_output(res)


if __name__ == "__main__":
    rng = np.random.default_rng(0)
    s = 1.0 / np.sqrt(D)
    inputs = dict(
        x=rng.standard_normal((B, N, D), dtype=np.float32),
        Wq=rng.standard_normal((D, D), dtype=np.float32) * s,
        bq=rng.standard_normal((D,), dtype=np.float32) * s,
        Wk=rng.standard_normal((D, D), dtype=np.float32) * s,
        bk=rng.standard_normal((D,), dtype=np.float32) * s,
        Wv=rng.standard_normal((D, D), dtype=np.float32) * s,
        bv=rng.standard_normal((D,), dtype=np.float32) * s,
        pos_bias=rng.standard_normal((N, N), dtype=np.float32) * 0.1,
    )
    out = kernel(**inputs)
    print("kernel ran, out shape:", out.shape)


# revision 5
# speedup vs baseline: 1.3042x; 1.3042x over previous
"""AFT-Full (Attention-Free Transformer) distributed Bass kernel for 8 TRN2 NeuronCores.

Reference math (B=4, N=512, D=128):
    q = sigmoid(x @ Wq^T + bq); k = x @ Wk^T + bk; v = x @ Wv^T + bv
    s[b,t,j,d] = k[b,j,d] + pos_bias[t,j];  m = max_j s
    out = q * (sum_j exp(s-m) * v) / (sum_j exp(s-m))

The max-stabilizer m cancels between numerator and denominator, and
exp(k + pb) = exp(pb) * exp(k), so with P = exp(pos_bias), ek = exp(k):
    out = q * (P @ (ek * v)) / (P @ ek)        (matmuls contract over j)
Further, sigmoid(q) / den = 1 / (den * (1 + exp(-qlin))), so the whole
epilogue needs only Exp activations and one fast reciprocal:
    out^T = num^T * recip(den^T + den^T * exp(-qlin^T))

Sharding: 8 cores = 4 batches x 2 t-halves; no collectives. Each core gets
x[b]^T with its t-half's columns rotated to the front, and pos_bias rows
rotated identically, so the j-contraction order matches and one SPMD graph
serves all cores. Device computes out^T[d, t] for its (b, t-half).

Compute dtype: bf16 operands into the PE array (fp32 PSUM accumulation) —
fp32 matmuls run ~4x slower on trn2. Biases are folded in as rank-1
matmuls accumulating into the same PSUM bank (free on the PE).
"""

import sys

import numpy as np

try:
    import concourse.bass as bass
except ImportError:  # pragma: no cover
    sys.path.insert(0, "/opt/trn_rl_repo")
    import concourse.bass as bass

import concourse.mybir as mybir
import concourse.tile as tile
from concourse import bacc
from concourse.bass_utils import run_bass_kernel_spmd

F32 = mybir.dt.float32
BF16 = mybir.dt.bfloat16
B, N, D = 4, 512, 128
T = N // 2  # t-rows per core
JT = N // 128  # j tiles of 128
AF = mybir.ActivationFunctionType


def build_nc() -> bass.Bass:
    # Bacc (not plain Bass): its compile() pass legalizes multi-wait
    # instructions (move_matmul_waits_to_ldweights, event semaphores),
    # which this walrus build requires.
    nc = bacc.Bacc()
    xT = nc.dram_tensor("xT", [D, N], F32, kind="ExternalInput")
    wall = nc.dram_tensor("wall", [D, 3 * D], F32, kind="ExternalInput")  # WqT|WkT|WvT
    ball = nc.dram_tensor("ball", [1, 3 * D], F32, kind="ExternalInput")  # bq|bk|bv
    pbT = nc.dram_tensor("pbT", [N, T], F32, kind="ExternalInput")
    out = nc.dram_tensor("out", [D, T], F32, kind="ExternalOutput")

    with tile.TileContext(nc) as tc:
        with (
            tc.tile_pool(name="sb", bufs=1) as sb,
            tc.tile_pool(name="ps", bufs=4, space="PSUM") as ps,
            tc.tile_pool(name="acc", bufs=1, space="PSUM") as acc,
        ):
            # ---- loads: xT/w/b on the SP HWDGE ring, pb on the ACT ring ----
            xT_sb = sb.tile([D, N], F32, name="xT_sb")
            nc.sync.dma_start(xT_sb[:], xT[:])
            w_sb = sb.tile([D, 3 * D], F32, name="w_sb")
            nc.sync.dma_start(w_sb[:], wall[:])
            b_sb = sb.tile([1, 3 * D], F32, name="b_sb")
            nc.sync.dma_start(b_sb[:], ball[:])
            pb_sb = sb.tile([128, JT, T], F32, name="pb_sb")
            nc.scalar.dma_start(pb_sb[:], pbT[:].rearrange("(j p) t -> p j t", p=128))

            # ---- bf16 casts ----
            xb = sb.tile([D, N], BF16, name="xb")
            nc.vector.tensor_copy(xb[:], xT_sb[:])
            wb = sb.tile([D, 3 * D], BF16, name="wb")
            nc.vector.tensor_copy(wb[:], w_sb[:])
            bb = sb.tile([1, 3 * D], BF16, name="bb")
            nc.vector.tensor_copy(bb[:], b_sb[:])
            ones = sb.tile([1, T], BF16, name="ones")
            nc.vector.memset(ones[:], 1.0)

            # ---- P^T tiles: exp(pos_bias^T) in bf16 ----
            pt = [sb.tile([128, T], BF16, name=f"pt{j}") for j in range(JT)]
            for j in range(JT):
                nc.scalar.activation(pt[j][:], pb_sb[:, j, :], AF.Exp)

            # ---- k/v projections per j-tile; PSUM = bias + x_j @ [Wk^T|Wv^T] ----
            ek = [sb.tile([128, D], BF16, name=f"ek{j}") for j in range(JT)]
            wt = [sb.tile([128, D], BF16, name=f"wt{j}") for j in range(JT)]
            for j in range(JT):
                kv_ps = ps.tile([128, 2 * D], F32, tag="kv_ps")
                nc.tensor.matmul(
                    kv_ps[:], ones[:, 0:128], bb[:, D : 3 * D], start=True, stop=False
                )
                nc.tensor.matmul(
                    kv_ps[:],
                    xb[:, j * 128 : (j + 1) * 128],
                    wb[:, D : 3 * D],
                    start=False,
                    stop=True,
                )
                nc.scalar.activation(ek[j][:], kv_ps[:, 0:D], AF.Exp)
                nc.vector.tensor_mul(wt[j][:], ek[j][:], kv_ps[:, D : 2 * D])

            # ---- q^T[d,t] = bq + Wq @ x[t-half]^T; then exp(-qlin) ----
            q_ps = acc.tile([D, T], F32, tag="q_ps")
            nc.tensor.matmul(q_ps[:], bb[:, 0:D], ones[:], start=True, stop=False)
            nc.tensor.matmul(q_ps[:], wb[:, 0:D], xb[:, 0:T], start=False, stop=True)
            eq = sb.tile([D, T], F32, name="eq")
            nc.scalar.activation(eq[:], q_ps[:], AF.Exp, scale=-1.0)

            # ---- den^T = sum_j ek_j @ pt_j ; num^T = sum_j wt_j @ pt_j ----
            den_ps = acc.tile([D, T], F32, tag="den_ps")
            num_ps = acc.tile([D, T], F32, tag="num_ps")
            for j in range(JT):
                nc.tensor.matmul(
                    den_ps[:], ek[j][:], pt[j][:],
                    start=(j == 0), stop=(j == JT - 1),
                )
            for j in range(JT):
                nc.tensor.matmul(
                    num_ps[:], wt[j][:], pt[j][:],
                    start=(j == 0), stop=(j == JT - 1),
                )

            # ---- out^T = num^T * recip(den^T * (1 + exp(-qlin^T))) ----
            t1 = sb.tile([D, T], F32, name="t1")
            nc.vector.tensor_mul(t1[:], eq[:], den_ps[:])
            f = sb.tile([D, T], F32, name="f")
            nc.vector.tensor_tensor(f[:], t1[:], den_ps[:], mybir.AluOpType.add)
            rec = sb.tile([D, T], F32, name="rec")
            nc.vector.reciprocal_approx_fast(rec[:], f[:])
            out_sb = sb.tile([D, T], F32, name="out_sb")
            nc.vector.tensor_mul(out_sb[:], rec[:], num_ps[:])
            nc.sync.dma_start(out[:], out_sb[:])

    nc.finalize()
    return nc


def prepare_in_maps(x, Wq, bq, Wk, bk, Wv, bv, pos_bias):
    x = np.asarray(x, dtype=np.float32)
    pos_bias = np.asarray(pos_bias, dtype=np.float32)
    wall = np.ascontiguousarray(
        np.concatenate(
            [
                np.asarray(Wq, np.float32).T,
                np.asarray(Wk, np.float32).T,
                np.asarray(Wv, np.float32).T,
            ],
            axis=1,
        )
    )
    ball = np.ascontiguousarray(
        np.concatenate(
            [np.asarray(bq, np.float32), np.asarray(bk, np.float32),
             np.asarray(bv, np.float32)]
        )[None]
    )

    in_maps = []
    for i in range(8):
        b, th = divmod(i, 2)
        t0 = th * T
        perm = np.concatenate([np.arange(t0, N), np.arange(0, t0)])
        in_maps.append(
            {
                "xT": np.ascontiguousarray(x[b][perm].T),
                "wall": wall,
                "ball": ball,
                "pbT": np.ascontiguousarray(pos_bias[t0 : t0 + T][:, perm].T),
            }
        )
    return in_maps


def assemble_output(results) -> np.ndarray:
    out = np.empty((B, N, D), np.float32)
    for i in range(8):
        b, th = divmod(i, 2)
        t0 = th * T
        out[b, t0 : t0 + T, :] = results[i]["out"].T
    return out


def kernel(x, Wq, bq, Wk, bk, Wv, bv, pos_bias) -> np.ndarray:
    in_maps = prepare_in_maps(x, Wq, bq, Wk, bk, Wv, bv, pos_bias)
    nc = build_nc()
    res = run_bass_kernel_spmd(nc, in_maps, core_ids=list(range(8))).results
    return assemble_output(res)


if __name__ == "__main__":
    rng = np.random.default_rng(0)
    s = 1.0 / np.sqrt(D)
    inputs = dict(
        x=rng.standard_normal((B, N, D), dtype=np.float32),
        Wq=rng.standard_normal((D, D), dtype=np.float32) * s,
        bq=rng.standard_normal((D,), dtype=np.float32) * s,
        Wk=rng.standard_normal((D, D), dtype=np.float32) * s,
        bk=rng.standard_normal((D,), dtype=np.float32) * s,
        Wv=rng.standard_normal((D, D), dtype=np.float32) * s,
        bv=rng.standard_normal((D,), dtype=np.float32) * s,
        pos_bias=rng.standard_normal((N, N), dtype=np.float32) * 0.1,
    )
    out = kernel(**inputs)
    print("kernel ran, out shape:", out.shape)


# revision 6
# speedup vs baseline: 1.5072x; 1.1557x over previous
"""AFT-Full (Attention-Free Transformer) distributed Bass kernel for 8 TRN2 NeuronCores.

Reference math (B=4, N=512, D=128):
    q = sigmoid(x @ Wq^T + bq); k = x @ Wk^T + bk; v = x @ Wv^T + bv
    s[b,t,j,d] = k[b,j,d] + pos_bias[t,j];  m = max_j s
    out = q * (sum_j exp(s-m) * v) / (sum_j exp(s-m))

The max-stabilizer m cancels between numerator and denominator, and
exp(k + pb) = exp(pb) * exp(k), so with P = exp(pos_bias), ek = exp(k):
    out = q * (P @ (ek * v)) / (P @ ek)        (matmuls contract over j)
Further, sigmoid(q)/den = 1/(den * (1 + exp(-qlin))), so the epilogue
needs only Exp activations (one ACT table) and one fast reciprocal:
    out^T = num^T * recip(den^T * (1 + exp(-qlin^T)))

Sharding: 8 cores = 4 batches x 2 t-halves; no collectives. Each core gets
x[b]^T with its t-half's columns rotated to the front, and pos_bias rows
rotated identically, so the j-contraction order matches and one SPMD graph
serves all cores. Device computes out^T[d, t] for its (b, t-half).

Compute dtype: bf16 into the PE array (fp32 PSUM accumulation) — fp32
matmuls run ~4x slower on trn2. f32->bf16 conversion happens inside the
SWDGE cast-DMAs on the otherwise-idle GpSimd engine. Biases are folded in
as rank-1 matmuls accumulating into the same PSUM bank (free on the PE).
"""

import sys

import numpy as np

try:
    import concourse.bass as bass
except ImportError:  # pragma: no cover
    sys.path.insert(0, "/opt/trn_rl_repo")
    import concourse.bass as bass

import concourse.mybir as mybir
import concourse.tile as tile
from concourse import bacc
from concourse.bass_utils import run_bass_kernel_spmd

F32 = mybir.dt.float32
BF16 = mybir.dt.bfloat16
B, N, D = 4, 512, 128
T = N // 2  # t-rows per core
JT = N // 128  # j tiles of 128
AF = mybir.ActivationFunctionType


def build_nc() -> bass.Bass:
    # Bacc (not plain Bass): its compile() pass legalizes multi-wait
    # instructions (move_matmul_waits_to_ldweights, event semaphores),
    # which this walrus build requires.
    nc = bacc.Bacc()
    # xw = [x[b]^T | Wq^T | Wk^T | Wv^T] packed on columns
    xw = nc.dram_tensor("xw", [D, N + 3 * D], F32, kind="ExternalInput")
    ball = nc.dram_tensor("ball", [1, 3 * D], F32, kind="ExternalInput")  # bq|bk|bv
    pbT = nc.dram_tensor("pbT", [N, T], F32, kind="ExternalInput")
    out = nc.dram_tensor("out", [D, T], F32, kind="ExternalOutput")

    with tile.TileContext(nc) as tc:
        with (
            tc.tile_pool(name="sb", bufs=1) as sb,
            tc.tile_pool(name="ps", bufs=1, space="PSUM") as ps,
        ):
            # ---- loads ----
            # biases + x/weights: SWDGE cast-DMA f32->bf16 on GpSimd
            bb = sb.tile([1, 3 * D], BF16, name="bb")
            nc.gpsimd.dma_start(bb[:], ball[:])
            xwb = sb.tile([D, N + 3 * D], BF16, name="xwb")
            nc.gpsimd.dma_start(xwb[:], xw[:])
            # pos_bias^T: HWDGE on the otherwise-free SP ring, f32
            pb_sb = sb.tile([128, JT, T], F32, name="pb_sb")
            nc.sync.dma_start(pb_sb[:], pbT[:].rearrange("(j p) t -> p j t", p=128))

            xb = xwb[:, 0:N]
            wb = xwb[:, N : N + 3 * D]
            ones = sb.tile([1, T], BF16, name="ones")
            nc.vector.memset(ones[:], 1.0)

            # ---- P^T = exp(pos_bias^T), one big ACT op -> bf16 ----
            pt = sb.tile([128, JT, T], BF16, name="pt")
            nc.scalar.activation(pt[:], pb_sb[:], AF.Exp)

            # ---- k/v projections: kv_all[:, j, 0:128]=k_j+bk, [...,128:256]=v_j+bv ----
            kv_all = ps.tile([128, JT, 2 * D], F32, tag="kv_all")
            for j in range(JT):
                nc.tensor.matmul(
                    kv_all[:, j, :], ones[:, 0:128], bb[:, D : 3 * D],
                    start=True, stop=False,
                )
                nc.tensor.matmul(
                    kv_all[:, j, :],
                    xb[:, j * 128 : (j + 1) * 128],
                    wb[:, D : 3 * D],
                    start=False, stop=True,
                )
            # q^T[d,t] = bq + Wq @ x[t-half]^T
            q_ps = ps.tile([D, T], F32, tag="q_ps")
            nc.tensor.matmul(q_ps[:], bb[:, 0:D], ones[:], start=True, stop=False)
            nc.tensor.matmul(q_ps[:], wb[:, 0:D], xb[:, 0:T], start=False, stop=True)

            # ---- batched exp(k) and ek*v ----
            ek = sb.tile([128, JT, D], BF16, name="ek")
            nc.scalar.activation(ek[:], kv_all[:, :, 0:D], AF.Exp)
            wt = sb.tile([128, JT, D], BF16, name="wt")
            nc.vector.tensor_mul(wt[:], ek[:], kv_all[:, :, D : 2 * D])
            # exp(-qlin), then g = 1 + exp(-qlin)  (off the critical tail)
            eq = sb.tile([D, T], F32, name="eq")
            nc.scalar.activation(eq[:], q_ps[:], AF.Exp, scale=-1.0)
            g = sb.tile([D, T], F32, name="g")
            nc.vector.tensor_scalar_add(g[:], eq[:], 1.0)

            # ---- den^T = sum_j ek_j @ pt_j ; num^T = sum_j wt_j @ pt_j ----
            den_ps = ps.tile([D, T], F32, tag="den_ps")
            num_ps = ps.tile([D, T], F32, tag="num_ps")
            for j in range(JT):
                nc.tensor.matmul(
                    den_ps[:], ek[:, j, :], pt[:, j, :],
                    start=(j == 0), stop=(j == JT - 1),
                )
            for j in range(JT):
                nc.tensor.matmul(
                    num_ps[:], wt[:, j, :], pt[:, j, :],
                    start=(j == 0), stop=(j == JT - 1),
                )

            # ---- out^T = num^T * recip(den^T * g), halved to overlap DMA-out ----
            f = sb.tile([D, T], F32, name="f")
            nc.vector.tensor_mul(f[:], g[:], den_ps[:])
            rec = sb.tile([D, T], F32, name="rec")
            nc.vector.reciprocal_approx_fast(rec[:], f[:])
            out_sb = sb.tile([D, T], F32, name="out_sb")
            half = T // 2
            nc.vector.tensor_mul(
                out_sb[:, 0:half], rec[:, 0:half], num_ps[:, 0:half]
            )
            nc.sync.dma_start(out[:, 0:half], out_sb[:, 0:half])
            nc.vector.tensor_mul(
                out_sb[:, half:T], rec[:, half:T], num_ps[:, half:T]
            )
            nc.sync.dma_start(out[:, half:T], out_sb[:, half:T])

    nc.finalize()
    return nc


def prepare_in_maps(x, Wq, bq, Wk, bk, Wv, bv, pos_bias):
    x = np.asarray(x, dtype=np.float32)
    pos_bias = np.asarray(pos_bias, dtype=np.float32)
    wall = np.concatenate(
        [
            np.asarray(Wq, np.float32).T,
            np.asarray(Wk, np.float32).T,
            np.asarray(Wv, np.float32).T,
        ],
        axis=1,
    )
    ball = np.ascontiguousarray(
        np.concatenate(
            [np.asarray(bq, np.float32), np.asarray(bk, np.float32),
             np.asarray(bv, np.float32)]
        )[None]
    )

    in_maps = []
    for i in range(8):
        b, th = divmod(i, 2)
        t0 = th * T
        perm = np.concatenate([np.arange(t0, N), np.arange(0, t0)])
        xw = np.ascontiguousarray(np.concatenate([x[b][perm].T, wall], axis=1))
        in_maps.append(
            {
                "xw": xw,
                "ball": ball,
                "pbT": np.ascontiguousarray(pos_bias[t0 : t0 + T][:, perm].T),
            }
        )
    return in_maps


def assemble_output(results) -> np.ndarray:
    out = np.empty((B, N, D), np.float32)
    for i in range(8):
        b, th = divmod(i, 2)
        t0 = th * T
        out[b, t0 : t0 + T, :] = results[i]["out"].T
    return out


def kernel(x, Wq, bq, Wk, bk, Wv, bv, pos_bias) -> np.ndarray:
    in_maps = prepare_in_maps(x, Wq, bq, Wk, bk, Wv, bv, pos_bias)
    nc = build_nc()
    res = run_bass_kernel_spmd(nc, in_maps, core_ids=list(range(8))).results
    return assemble_output(res)


if __name__ == "__main__":
    rng = np.random.default_rng(0)
    s = 1.0 / np.sqrt(D)
    inputs = dict(
        x=rng.standard_normal((B, N, D), dtype=np.float32),
        Wq=rng.standard_normal((D, D), dtype=np.float32) * s,
        bq=rng.standard_normal((D,), dtype=np.float32) * s,
        Wk=rng.standard_normal((D, D), dtype=np.float32) * s,
        bk=rng.standard_normal((D,), dtype=np.float32) * s,
        Wv=rng.standard_normal((D, D), dtype=np.float32) * s,
        bv=rng.standard_normal((D,), dtype=np.float32) * s,
        pos_bias=rng.standard_normal((N, N), dtype=np.float32) * 0.1,
    )
    out = kernel(**inputs)
    print("kernel ran, out shape:", out.shape)
